# revision 22
# baseline (speedup 1.0000x reference)
"""Trainium2 Bass kernel for CropPoolLayer (TF crop_and_resize bilinear + 2x2 maxpool).

Decomposition (host precomputes gather indices + expanded bilinear weights):
  per ROI: crops[c, (di,dj,pi,pj)] = sum_q G[q, c] * W[q, (di,dj,pi,pj)]
  pooled[c, (pi,pj)] = max over (di,dj)

Device (per core, SPMD over 8 cores, 64 ROIs each), channel-major layout:
  - ROI slots are bin-packed into 128-row gather blocks (first-fit
    decreasing, big slots split into full blocks + remainder placed after)
    so most slots need exactly one [128,196] weight piece / 4 matmuls,
    minimizing both PE matmul count and expanded-weight DMA bytes.
  - one merged dma_gather per chunk of 8 blocks (fp16 feature rows, 1KB
    each); per piece: 4 matmuls lhsT = gathered [128 pts, 128 c], rhs =
    fp16 weights [128, 196], accumulating into PSUM [128 c, 4, 196]
    (cols ordered (di,dj,pi,pj))
  - pool paths rotate across engines to balance load:
      D : DVE max(psum di0, psum di1) -> f16, DVE max dj halves
      AP: ACT evacuates all 196 cols to f16, Pool (gpsimd) folds di+dj
      AD: ACT evacuates, DVE folds di+dj (4x f16 mode)
  - fp16 outputs DMA'd per 2 ROI pairs; host converts to f32 and unshards

Slot structure is shared across all 8 cores (SPMD, one program): ROIs are
rank-sorted by gathered-point count; slot (completion index) s takes rank
octet group[s], so per-slot piece decomposition is built from the max
count over cores.
"""
import sys

sys.path.insert(0, "/opt/trn_rl_repo")

import numpy as np

POOL = 7
CROP = 14
B, H, W, C = 2, 64, 64, 512
NROI = 512
NCORES = 8
NR = NROI // NCORES           # 64 ROIs per core
NGROUP = NR                   # rank-octet groups (one per slot)
MCOLS = 196                   # (di,dj,pi,pj) weight columns
BINCAP = 128                  # gather block rows (matmul contraction dim)
BINS_PER_CHUNK = 8            # gather blocks per dma_gather
# pooling path per completed slot, cycled: D = DVE-direct, AP = ACT+Pool,
# AD = ACT+DVE  (balances ACT/DVE/Pool engine busy)
PATH_CYCLE = ("AD", "AD", "TR")

# col = di*98 + dj*49 + pi*7 + pj  ->  crop row (2pi+di)*14 + (2pj+dj)
_COLMAP = np.empty(MCOLS, np.int64)
for _di in range(2):
    for _dj in range(2):
        for _pi in range(POOL):
            for _pj in range(POOL):
                _COLMAP[_di * 98 + _dj * 49 + _pi * 7 + _pj] = \
                    (2 * _pi + _di) * CROP + (2 * _pj + _dj)


def _grid_geometry(rois, im_info):
    rois = np.asarray(rois, dtype=np.float32)
    im_h = np.float32(im_info[0])
    im_w = np.float32(im_info[1])
    bids = rois[:, 0].astype(np.int32)
    x1 = rois[:, 1] / im_w
    y1 = rois[:, 2] / im_h
    x2 = rois[:, 3] / im_w
    y2 = rois[:, 4] / im_h
    grid = np.arange(CROP, dtype=np.float32)
    ys = y1[:, None] * np.float32(H - 1) + grid[None, :] * ((y2 - y1) * np.float32(H - 1) / np.float32(CROP - 1))[:, None]
    xs = x1[:, None] * np.float32(W - 1) + grid[None, :] * ((x2 - x1) * np.float32(W - 1) / np.float32(CROP - 1))[:, None]
    vy = (ys >= 0) & (ys <= H - 1)
    vx = (xs >= 0) & (xs <= W - 1)
    y0f = np.floor(ys)
    x0f = np.floor(xs)
    ly = (ys - y0f).astype(np.float32)
    lx = (xs - x0f).astype(np.float32)
    y0 = np.clip(y0f, 0, H - 1).astype(np.int32)
    y1i = np.clip(np.ceil(ys), 0, H - 1).astype(np.int32)
    x0 = np.clip(x0f, 0, W - 1).astype(np.int32)
    x1i = np.clip(np.ceil(xs), 0, W - 1).astype(np.int32)
    return dict(bids=bids, ly=ly, lx=lx, y0=y0, y1i=y1i, x0=x0, x1i=x1i, vy=vy, vx=vx)


def _roi_weights(g, n):
    y0 = g["y0"][n]; y1i = g["y1i"][n]; x0 = g["x0"][n]; x1i = g["x1i"][n]
    ly = g["ly"][n]; lx = g["lx"][n]; vy = g["vy"][n]; vx = g["vx"][n]
    rmin = int(min(y0.min(), y1i.min())); rmax = int(max(y0.max(), y1i.max()))
    cmin = int(min(x0.min(), x1i.min())); cmax = int(max(x0.max(), x1i.max()))
    nrows = rmax - rmin + 1; ncols = cmax - cmin + 1
    Wy = np.zeros((CROP, nrows), np.float32)
    Wx = np.zeros((CROP, ncols), np.float32)
    ii = np.arange(CROP)
    np.add.at(Wy, (ii, y0 - rmin), ((1.0 - ly) * vy).astype(np.float32))
    np.add.at(Wy, (ii, y1i - rmin), (ly * vy).astype(np.float32))
    np.add.at(Wx, (ii, x0 - cmin), ((1.0 - lx) * vx).astype(np.float32))
    np.add.at(Wx, (ii, x1i - cmin), (lx * vx).astype(np.float32))
    Wfull = np.einsum("ir,jx->ijrx", Wy, Wx).reshape(CROP * CROP, nrows * ncols)
    return int(g["bids"][n]), rmin, cmin, nrows, ncols, Wfull


def _roi_pts(g):
    pts = np.empty(NROI, np.int64)
    for n in range(NROI):
        y0 = g["y0"][n]; y1i = g["y1i"][n]; x0 = g["x0"][n]; x1i = g["x1i"][n]
        nrows = int(max(y0.max(), y1i.max())) - int(min(y0.min(), y1i.min())) + 1
        ncols = int(max(x0.max(), x1i.max())) - int(min(x0.min(), x1i.min())) + 1
        pts[n] = nrows * ncols
    return pts


def _plan_structure(sizes):
    """Bin-pack the 64 group sizes into 128-row gather blocks.

    First-fit decreasing; groups > 128 emit dedicated full blocks, and the
    remainder is placed only into blocks AFTER the last full one so each
    group's pieces are consecutive in emission order (keeps a single PSUM
    accumulation open at a time and pieces within one gather chunk).

    Returns (bins, grp_pieces, slot_order):
      bins[b]       = list of (grp, off, ln)
      grp_pieces[g] = list of (bin, off, ln, pcol) in emission order
      slot_order    = groups ordered by completion (emission order)
    """
    order_g = sorted(range(NGROUP), key=lambda gg: -sizes[gg])
    bins = []                                  # [fill, [(grp, off, ln)]]
    gp = {gg: [] for gg in range(NGROUP)}
    for gg in order_g:
        sz = int(sizes[gg]); lastfull = -1
        while sz > BINCAP:
            gp[gg].append((len(bins), 0, BINCAP))
            bins.append([BINCAP, [(gg, 0, BINCAP)]])
            lastfull = len(bins) - 1
            sz -= BINCAP
        if sz:
            for bi in range(lastfull + 1, len(bins)):
                if BINCAP - bins[bi][0] >= sz:
                    gp[gg].append((bi, bins[bi][0], sz))
                    bins[bi][1].append((gg, bins[bi][0], sz))
                    bins[bi][0] += sz
                    break
            else:
                gp[gg].append((len(bins), 0, sz))
                bins.append([sz, [(gg, 0, sz)]])
    # assign pcol in emission order (bin-major, pieces in bin order)
    pcol = 0
    emit = []                                  # (bin, grp, off, ln) in order
    pcol_of = {}
    for bi, (_fill, pieces) in enumerate(bins):
        for (gg, off, ln) in pieces:
            pcol_of[(bi, gg, off)] = pcol
            emit.append((bi, gg, off, ln))
            pcol += 1
    for gg in range(NGROUP):
        gp[gg] = [(bi, off, ln, pcol_of[(bi, gg, off)]) for (bi, off, ln) in gp[gg]]
    # completion order of groups = emission index of each group's LAST piece
    last_emit = {}
    for i, (bi, gg, off, ln) in enumerate(emit):
        last_emit[gg] = i
    slot_order = sorted(range(NGROUP), key=lambda gg: last_emit[gg])
    return [b[1] for b in bins], gp, slot_order


def _build_core_plan(g, roi_of_group, grp_pieces, nbins, totp):
    """Per-core gather indices + expanded weights for the shared structure."""
    rows = nbins * BINCAP
    gidx = np.zeros(rows, np.int16)            # holes gather row 0 (zero weights)
    wts = np.zeros((BINCAP, totp, MCOLS), np.float16)
    for gg, n in enumerate(roi_of_group):
        bid, rmin, cmin, nrows, ncols, Wfull = _roi_weights(g, n)
        pts = nrows * ncols
        rr, xx = np.divmod(np.arange(pts), ncols)
        flat_idx = (bid * (H * W) + (rmin + rr) * W + (cmin + xx)).astype(np.int16)
        Wc = Wfull[_COLMAP].astype(np.float16)          # [196, pts]
        cur = 0
        for (bi, off, ln, pcol) in grp_pieces[gg]:
            k = min(ln, max(0, pts - cur))              # this core may have fewer pts
            if k > 0:
                gpos = bi * BINCAP + off
                gidx[gpos: gpos + k] = flat_idx[cur: cur + k]
                wts[off: off + k, pcol, :] = Wc[:, cur: cur + k].T
            cur += ln
    it = gidx.reshape(-1, 16).T.copy()                  # [16, rows//16]
    return dict(gidx=np.tile(it, (8, 1)), w=np.ascontiguousarray(wts.reshape(BINCAP, totp * MCOLS)))


_NC_CACHE = {}


def _build_bass(bins, grp_pieces, slot_order):
    key = (tuple(tuple(p) for b in bins for p in b), tuple(slot_order))
    if key in _NC_CACHE:
        return _NC_CACHE[key]
    import concourse.bacc as bacc
    import concourse.mybir as mybir
    from concourse.tile import TileContext

    f16 = mybir.dt.float16
    f32 = mybir.dt.float32
    nbins = len(bins)
    totp = sum(len(v) for v in grp_pieces.values())
    rows = nbins * BINCAP
    idx_cols = rows // 16
    slot_of = {gg: s for s, gg in enumerate(slot_order)}

    nc = bacc.Bacc("TRN2", target_bir_lowering=False, debug=False,
                   num_devices=NCORES, dynamic_dma_scratch_size=131072)
    bottom = nc.dram_tensor("bottom", [B * H * W, C], f16, kind="ExternalInput")
    gidx = nc.dram_tensor("gidx", [128, idx_cols], mybir.dt.int16, kind="ExternalInput")
    wts = nc.dram_tensor("w", [128, totp * MCOLS], f16, kind="ExternalInput")
    out = nc.dram_tensor("out", [NR // 2, 128, 2 * 4 * 49], f16, kind="ExternalOutput")

    mmax = mybir.AluOpType.max
    # small leading chunks shorten the pipeline fill (first matmul waits on
    # the first gather's prep + transfer)
    splits = []
    b0 = 0
    for sz in (2, 6):
        if b0 < nbins:
            splits.append((b0, min(b0 + sz, nbins)))
            b0 += sz
    while b0 < nbins:
        splits.append((b0, min(b0 + BINS_PER_CHUNK, nbins)))
        b0 += BINS_PER_CHUNK
    chunks = [list(range(a, b)) for a, b in splits]
    chunk_of_bin = {}
    for ci, bl in enumerate(chunks):
        for bi in bl:
            chunk_of_bin[bi] = (ci, bi - bl[0])
    for gg, pieces in grp_pieces.items():
        cis = [chunk_of_bin[p[0]][0] for p in pieces]
        assert max(cis) - min(cis) <= 1, f"group {gg} spans >2 chunks: {cis}"
    # pieces per chunk in pcol order (pcol was assigned bin-major)
    first_pcol = {}
    npieces_chunk = [0] * len(chunks)
    pcol = 0
    for bi in range(nbins):
        ci = chunk_of_bin[bi][0]
        for _ in bins[bi]:
            first_pcol.setdefault(ci, pcol)
            npieces_chunk[ci] += 1
            pcol += 1

    with TileContext(nc) as tc:
        with (
            tc.tile_pool(name="idxp", bufs=1) as idxp,
            tc.tile_pool(name="gp", bufs=4) as gp,
            tc.tile_pool(name="wp", bufs=4) as wp,
            tc.tile_pool(name="sbp", bufs=8) as sbp,
            tc.tile_pool(name="fp", bufs=4) as fp,
            tc.tile_pool(name="psp", bufs=4, space="PSUM") as psp,
        ):
            it = idxp.tile([128, idx_cols], mybir.dt.int16, tag="idx")
            nc.sync.dma_start(out=it[:], in_=gidx[:])

            ndone = 0                   # completed slots
            f2 = None
            gc_of = {}                  # chunk -> (gc tile, first bin)
            wt_of = {}                  # chunk -> (wt tile, first pcol)
            # issue ALL gathers (Pool queue) and ALL weight copies (SP queue)
            # up front in program order: each queue then only ever waits on
            # its own tile-pool rotation, never behind out-DMAs (SP) or
            # compute (head-of-line blocking)
            for ci, bl in enumerate(chunks):
                nb = len(bl)
                gc = gp.tile([128, nb, C], f16, tag="gt")
                gc_of[ci] = (gc, bl[0])
                pos0 = bl[0] * BINCAP
                nc.gpsimd.dma_gather(
                    out_ap=gc[:], in_ap=bottom[:],
                    idxs_ap=it[:, pos0 // 16: (pos0 + nb * BINCAP) // 16],
                    num_idxs=nb * BINCAP, num_idxs_reg=nb * BINCAP,
                    elem_size=C,
                )
            for ci, bl in enumerate(chunks):
                np_c = npieces_chunk[ci]
                pc0 = first_pcol[ci]
                wt = wp.tile([128, np_c, MCOLS], f16, tag="wt")
                wt_of[ci] = (wt, pc0)
                nh = max(1, np_c // (4 if ci == 0 else 2))
                for h0 in range(0, np_c, nh):
                    h1 = min(h0 + nh, np_c)
                    nc.sync.dma_start(
                        out=wt[:, h0:h1],
                        in_=wts[:, (pc0 + h0) * MCOLS: (pc0 + h1) * MCOLS]
                        .rearrange("p (n m) -> p n m", m=MCOLS),
                    )
            for ci, bl in enumerate(chunks):
                for bi in bl:
                    for (gg, off, ln) in bins[bi]:
                        pieces = grp_pieces[gg]
                        if (bi, off) != (pieces[-1][0], pieces[-1][1]):
                            continue    # emit at the group's LAST piece
                        # q-major so each PSUM accumulation group (q-region)
                        # is consecutive — interleaved groups corrupt the PE
                        ps = psp.tile([128, 4, 256], f32, tag="ps")
                        for q in range(4):
                            for pi, (bi2, off2, ln2, pcol2) in enumerate(pieces):
                                ci2 = chunk_of_bin[bi2][0]
                                gc2, bl02 = gc_of[ci2]
                                wt2, pc02 = wt_of[ci2]
                                nc.tensor.matmul(
                                    out=ps[:, q, 0:MCOLS],
                                    lhsT=gc2[:, bi2 - bl02, q * 128: (q + 1) * 128],
                                    rhs=wt2[:, pcol2 - pc02, :],
                                    start=(pi == 0), stop=(pi == len(pieces) - 1),
                                )
                        # slot complete -> pool
                        i = ndone
                        ndone += 1
                        if i % 4 == 0:
                            f2 = fp.tile([128, 2, 2, 4, 49], f16, tag="f2")
                        fslice = f2[:, (i % 4) // 2, i % 2]
                        path = PATH_CYCLE[i % len(PATH_CYCLE)]
                        if path == "TR":
                            # single DVE op: (di,dj) cols sit at stride 49,
                            # reduce the innermost [4] window psum -> f16
                            nc.vector.tensor_reduce(
                                out=fslice,
                                in_=ps[:, :, 0:196].rearrange(
                                    "p q (d s) -> p q s d", d=4),
                                axis=mybir.AxisListType.X, op=mmax)
                        else:
                            # ACT evacuates all 196 cols; DVE folds in 4x f16
                            vt = sbp.tile([128, 4, 2, 98], f16, tag="vt")
                            nc.scalar.copy(out=vt[:], in_=ps[:, :, 0:196])
                            ft1 = sbp.tile([128, 4, 98], f16, tag="ft1")
                            nc.vector.tensor_tensor(
                                out=ft1[:], in0=vt[:, :, 0], in1=vt[:, :, 1], op=mmax)
                            nc.vector.tensor_tensor(
                                out=fslice, in0=ft1[:, :, 0:49], in1=ft1[:, :, 49:98], op=mmax)
                        if i % 4 == 3:
                            p0 = (i - 3) // 2
                            nc.sync.dma_start(
                                out=out[p0: p0 + 2]
                                .rearrange("g p (s q i) -> p g s q i", s=2, q=4),
                                in_=f2[:],
                            )
    nc.compile()
    _NC_CACHE[key] = nc
    return nc


def _prepare(bottom, rois, im_info):
    g = _grid_geometry(rois, im_info)
    pts = _roi_pts(g)
    order = np.argsort(pts, kind="stable")
    sizes = [int(pts[order[8 * gg + 7]]) for gg in range(NGROUP)]
    bins, grp_pieces, slot_order = _plan_structure(sizes)
    totp = sum(len(v) for v in grp_pieces.values())
    # cores[c][slot] = ROI id; slot s <-> group slot_order[s]
    cores = [[int(order[8 * gg + c]) for gg in slot_order] for c in range(NCORES)]
    flat16 = np.ascontiguousarray(np.asarray(bottom, np.float32).reshape(B * H * W, C).astype(np.float16))
    in_maps = []
    for c in range(NCORES):
        roi_of_group = [int(order[8 * gg + c]) for gg in range(NGROUP)]
        p = _build_core_plan(g, roi_of_group, grp_pieces, len(bins), totp)
        in_maps.append({"bottom": flat16, "gidx": p["gidx"], "w": p["w"]})
    return cores, bins, grp_pieces, slot_order, in_maps


def kernel(bottom, rois, im_info):
    from concourse.bass_utils import run_bass_kernel_spmd

    cores, bins, grp_pieces, slot_order, in_maps = _prepare(bottom, rois, im_info)
    nc = _build_bass(bins, grp_pieces, slot_order)
    res = run_bass_kernel_spmd(nc, in_maps, core_ids=list(range(NCORES)))
    out = np.empty((NROI, POOL, POOL, C), np.float32)
    for c in range(NCORES):
        r = res.results[c]["out"].reshape(NR // 2, 128, 2, 4, 7, 7).astype(np.float32)
        # [pair, c128, sl, q, pi, pj] -> [slot, pi, pj, q*128+c128]
        r = r.transpose(0, 2, 4, 5, 3, 1).reshape(NR, POOL, POOL, C)
        out[np.array(cores[c])] = r
    return out


# revision 23
# speedup vs baseline: 1.0684x; 1.0684x over previous
"""Trainium2 Bass kernel for CropPoolLayer (TF crop_and_resize bilinear + 2x2 maxpool).

Decomposition (host precomputes gather indices + expanded bilinear weights):
  per ROI: crops[c, (di,dj,pi,pj)] = sum_q G[q, c] * W[q, (di,dj,pi,pj)]
  pooled[c, (pi,pj)] = max over (di,dj)

Device (per core, SPMD over 8 cores, 64 ROIs each), channel-major layout:
  - ROI slots are bin-packed into 128-row gather blocks (first-fit
    decreasing, big slots split into full blocks + remainder placed after)
    so most slots need exactly one [128,196] weight piece / 4 matmuls,
    minimizing both PE matmul count and expanded-weight DMA bytes.
  - one merged dma_gather per chunk of 8 blocks (fp16 feature rows, 1KB
    each); per piece: 4 matmuls lhsT = gathered [128 pts, 128 c], rhs =
    fp16 weights [128, 196], accumulating into PSUM [128 c, 4, 196]
    (cols ordered (di,dj,pi,pj))
  - pool paths rotate across engines to balance load:
      D : DVE max(psum di0, psum di1) -> f16, DVE max dj halves
      AP: ACT evacuates all 196 cols to f16, Pool (gpsimd) folds di+dj
      AD: ACT evacuates, DVE folds di+dj (4x f16 mode)
  - fp16 outputs DMA'd per 2 ROI pairs; host converts to f32 and unshards

Slot structure is shared across all 8 cores (SPMD, one program): ROIs are
rank-sorted by gathered-point count; slot (completion index) s takes rank
octet group[s], so per-slot piece decomposition is built from the max
count over cores.
"""
import sys

sys.path.insert(0, "/opt/trn_rl_repo")

import numpy as np

POOL = 7
CROP = 14
B, H, W, C = 2, 64, 64, 512
NROI = 512
NCORES = 8
NR = NROI // NCORES           # 64 ROIs per core
NGROUP = NR                   # rank-octet groups (one per slot)
MCOLS = 196                   # (di,dj,pi,pj) weight columns
BINCAP = 128                  # gather block rows (matmul contraction dim)
BINS_PER_CHUNK = 4            # gather blocks per dma_gather
# pooling path per completed slot, cycled: D = DVE-direct, AP = ACT+Pool,
# AD = ACT+DVE  (balances ACT/DVE/Pool engine busy)
PATH_CYCLE = ("AD", "AD", "TR")

# col = di*98 + dj*49 + pi*7 + pj  ->  crop row (2pi+di)*14 + (2pj+dj)
_COLMAP = np.empty(MCOLS, np.int64)
for _di in range(2):
    for _dj in range(2):
        for _pi in range(POOL):
            for _pj in range(POOL):
                _COLMAP[_di * 98 + _dj * 49 + _pi * 7 + _pj] = \
                    (2 * _pi + _di) * CROP + (2 * _pj + _dj)


def _grid_geometry(rois, im_info):
    rois = np.asarray(rois, dtype=np.float32)
    im_h = np.float32(im_info[0])
    im_w = np.float32(im_info[1])
    bids = rois[:, 0].astype(np.int32)
    x1 = rois[:, 1] / im_w
    y1 = rois[:, 2] / im_h
    x2 = rois[:, 3] / im_w
    y2 = rois[:, 4] / im_h
    grid = np.arange(CROP, dtype=np.float32)
    ys = y1[:, None] * np.float32(H - 1) + grid[None, :] * ((y2 - y1) * np.float32(H - 1) / np.float32(CROP - 1))[:, None]
    xs = x1[:, None] * np.float32(W - 1) + grid[None, :] * ((x2 - x1) * np.float32(W - 1) / np.float32(CROP - 1))[:, None]
    vy = (ys >= 0) & (ys <= H - 1)
    vx = (xs >= 0) & (xs <= W - 1)
    y0f = np.floor(ys)
    x0f = np.floor(xs)
    ly = (ys - y0f).astype(np.float32)
    lx = (xs - x0f).astype(np.float32)
    y0 = np.clip(y0f, 0, H - 1).astype(np.int32)
    y1i = np.clip(np.ceil(ys), 0, H - 1).astype(np.int32)
    x0 = np.clip(x0f, 0, W - 1).astype(np.int32)
    x1i = np.clip(np.ceil(xs), 0, W - 1).astype(np.int32)
    return dict(bids=bids, ly=ly, lx=lx, y0=y0, y1i=y1i, x0=x0, x1i=x1i, vy=vy, vx=vx)


def _roi_weights(g, n):
    y0 = g["y0"][n]; y1i = g["y1i"][n]; x0 = g["x0"][n]; x1i = g["x1i"][n]
    ly = g["ly"][n]; lx = g["lx"][n]; vy = g["vy"][n]; vx = g["vx"][n]
    rmin = int(min(y0.min(), y1i.min())); rmax = int(max(y0.max(), y1i.max()))
    cmin = int(min(x0.min(), x1i.min())); cmax = int(max(x0.max(), x1i.max()))
    nrows = rmax - rmin + 1; ncols = cmax - cmin + 1
    Wy = np.zeros((CROP, nrows), np.float32)
    Wx = np.zeros((CROP, ncols), np.float32)
    ii = np.arange(CROP)
    np.add.at(Wy, (ii, y0 - rmin), ((1.0 - ly) * vy).astype(np.float32))
    np.add.at(Wy, (ii, y1i - rmin), (ly * vy).astype(np.float32))
    np.add.at(Wx, (ii, x0 - cmin), ((1.0 - lx) * vx).astype(np.float32))
    np.add.at(Wx, (ii, x1i - cmin), (lx * vx).astype(np.float32))
    Wfull = np.einsum("ir,jx->ijrx", Wy, Wx).reshape(CROP * CROP, nrows * ncols)
    return int(g["bids"][n]), rmin, cmin, nrows, ncols, Wfull


def _roi_pts(g):
    pts = np.empty(NROI, np.int64)
    for n in range(NROI):
        y0 = g["y0"][n]; y1i = g["y1i"][n]; x0 = g["x0"][n]; x1i = g["x1i"][n]
        nrows = int(max(y0.max(), y1i.max())) - int(min(y0.min(), y1i.min())) + 1
        ncols = int(max(x0.max(), x1i.max())) - int(min(x0.min(), x1i.min())) + 1
        pts[n] = nrows * ncols
    return pts


def _plan_structure(sizes):
    """Bin-pack the 64 group sizes into 128-row gather blocks.

    First-fit decreasing; groups > 128 emit dedicated full blocks, and the
    remainder is placed only into blocks AFTER the last full one so each
    group's pieces are consecutive in emission order (keeps a single PSUM
    accumulation open at a time and pieces within one gather chunk).

    Returns (bins, grp_pieces, slot_order):
      bins[b]       = list of (grp, off, ln)
      grp_pieces[g] = list of (bin, off, ln, pcol) in emission order
      slot_order    = groups ordered by completion (emission order)
    """
    order_g = sorted(range(NGROUP), key=lambda gg: -sizes[gg])
    bins = []                                  # [fill, [(grp, off, ln)]]
    gp = {gg: [] for gg in range(NGROUP)}
    for gg in order_g:
        sz = int(sizes[gg]); lastfull = -1
        while sz > BINCAP:
            gp[gg].append((len(bins), 0, BINCAP))
            bins.append([BINCAP, [(gg, 0, BINCAP)]])
            lastfull = len(bins) - 1
            sz -= BINCAP
        if sz:
            for bi in range(lastfull + 1, len(bins)):
                if BINCAP - bins[bi][0] >= sz:
                    gp[gg].append((bi, bins[bi][0], sz))
                    bins[bi][1].append((gg, bins[bi][0], sz))
                    bins[bi][0] += sz
                    break
            else:
                gp[gg].append((len(bins), 0, sz))
                bins.append([sz, [(gg, 0, sz)]])
    # assign pcol in emission order (bin-major, pieces in bin order)
    pcol = 0
    emit = []                                  # (bin, grp, off, ln) in order
    pcol_of = {}
    for bi, (_fill, pieces) in enumerate(bins):
        for (gg, off, ln) in pieces:
            pcol_of[(bi, gg, off)] = pcol
            emit.append((bi, gg, off, ln))
            pcol += 1
    for gg in range(NGROUP):
        gp[gg] = [(bi, off, ln, pcol_of[(bi, gg, off)]) for (bi, off, ln) in gp[gg]]
    # completion order of groups = emission index of each group's LAST piece
    last_emit = {}
    for i, (bi, gg, off, ln) in enumerate(emit):
        last_emit[gg] = i
    slot_order = sorted(range(NGROUP), key=lambda gg: last_emit[gg])
    return [b[1] for b in bins], gp, slot_order


def _build_core_plan(g, roi_of_group, grp_pieces, nbins, totp):
    """Per-core gather indices + expanded weights for the shared structure."""
    rows = nbins * BINCAP
    gidx = np.zeros(rows, np.int16)            # holes gather row 0 (zero weights)
    wts = np.zeros((BINCAP, totp, MCOLS), np.float16)
    for gg, n in enumerate(roi_of_group):
        bid, rmin, cmin, nrows, ncols, Wfull = _roi_weights(g, n)
        pts = nrows * ncols
        rr, xx = np.divmod(np.arange(pts), ncols)
        flat_idx = (bid * (H * W) + (rmin + rr) * W + (cmin + xx)).astype(np.int16)
        Wc = Wfull[_COLMAP].astype(np.float16)          # [196, pts]
        cur = 0
        for (bi, off, ln, pcol) in grp_pieces[gg]:
            k = min(ln, max(0, pts - cur))              # this core may have fewer pts
            if k > 0:
                gpos = bi * BINCAP + off
                gidx[gpos: gpos + k] = flat_idx[cur: cur + k]
                wts[off: off + k, pcol, :] = Wc[:, cur: cur + k].T
            cur += ln
    it = gidx.reshape(-1, 16).T.copy()                  # [16, rows//16]
    return dict(gidx=np.tile(it, (8, 1)), w=np.ascontiguousarray(wts.reshape(BINCAP, totp * MCOLS)))


_NC_CACHE = {}


def _build_bass(bins, grp_pieces, slot_order):
    key = (tuple(tuple(p) for b in bins for p in b), tuple(slot_order))
    if key in _NC_CACHE:
        return _NC_CACHE[key]
    import concourse.bacc as bacc
    import concourse.mybir as mybir
    from concourse.tile import TileContext

    f16 = mybir.dt.float16
    f32 = mybir.dt.float32
    nbins = len(bins)
    totp = sum(len(v) for v in grp_pieces.values())
    rows = nbins * BINCAP
    idx_cols = rows // 16
    slot_of = {gg: s for s, gg in enumerate(slot_order)}

    nc = bacc.Bacc("TRN2", target_bir_lowering=False, debug=False,
                   num_devices=NCORES, dynamic_dma_scratch_size=131072)
    bottom = nc.dram_tensor("bottom", [B * H * W, C], f16, kind="ExternalInput")
    gidx = nc.dram_tensor("gidx", [128, idx_cols], mybir.dt.int16, kind="ExternalInput")
    wts = nc.dram_tensor("w", [128, totp * MCOLS], f16, kind="ExternalInput")
    out = nc.dram_tensor("out", [NR // 2, 128, 2 * 4 * 49], f16, kind="ExternalOutput")

    mmax = mybir.AluOpType.max
    # small leading chunks shorten the pipeline fill (first matmul waits on
    # the first gather's prep + transfer)
    splits = []
    b0 = 0
    for sz in (2, 6):
        if b0 < nbins:
            splits.append((b0, min(b0 + sz, nbins)))
            b0 += sz
    while b0 < nbins:
        splits.append((b0, min(b0 + BINS_PER_CHUNK, nbins)))
        b0 += BINS_PER_CHUNK
    chunks = [list(range(a, b)) for a, b in splits]
    chunk_of_bin = {}
    for ci, bl in enumerate(chunks):
        for bi in bl:
            chunk_of_bin[bi] = (ci, bi - bl[0])
    for gg, pieces in grp_pieces.items():
        cis = [chunk_of_bin[p[0]][0] for p in pieces]
        assert max(cis) - min(cis) <= 1, f"group {gg} spans >2 chunks: {cis}"
    # pieces per chunk in pcol order (pcol was assigned bin-major)
    first_pcol = {}
    npieces_chunk = [0] * len(chunks)
    pcol = 0
    for bi in range(nbins):
        ci = chunk_of_bin[bi][0]
        for _ in bins[bi]:
            first_pcol.setdefault(ci, pcol)
            npieces_chunk[ci] += 1
            pcol += 1

    with TileContext(nc) as tc:
        with (
            tc.tile_pool(name="idxp", bufs=1) as idxp,
            tc.tile_pool(name="gp", bufs=4) as gp,
            tc.tile_pool(name="wp", bufs=4) as wp,
            tc.tile_pool(name="sbp", bufs=8) as sbp,
            tc.tile_pool(name="fp", bufs=4) as fp,
            tc.tile_pool(name="psp", bufs=4, space="PSUM") as psp,
        ):
            it = idxp.tile([128, idx_cols], mybir.dt.int16, tag="idx")
            nc.sync.dma_start(out=it[:], in_=gidx[:])

            ndone = 0                   # completed slots
            f2 = None
            gc_of = {}                  # chunk -> (gc tile, first bin)
            wt_of = {}                  # chunk -> (wt tile, first pcol)
            # issue ALL gathers (Pool queue) and ALL weight copies (SP queue)
            # up front in program order: each queue then only ever waits on
            # its own tile-pool rotation, never behind out-DMAs (SP) or
            # compute (head-of-line blocking)
            for ci, bl in enumerate(chunks):
                nb = len(bl)
                gc = gp.tile([128, nb, C], f16, tag="gt")
                gc_of[ci] = (gc, bl[0])
                pos0 = bl[0] * BINCAP
                nc.gpsimd.dma_gather(
                    out_ap=gc[:], in_ap=bottom[:],
                    idxs_ap=it[:, pos0 // 16: (pos0 + nb * BINCAP) // 16],
                    num_idxs=nb * BINCAP, num_idxs_reg=nb * BINCAP,
                    elem_size=C,
                )
            for ci, bl in enumerate(chunks):
                np_c = npieces_chunk[ci]
                pc0 = first_pcol[ci]
                wt = wp.tile([128, np_c, MCOLS], f16, tag="wt")
                wt_of[ci] = (wt, pc0)
                nh = max(1, np_c // (4 if ci == 0 else 2))
                for h0 in range(0, np_c, nh):
                    h1 = min(h0 + nh, np_c)
                    nc.sync.dma_start(
                        out=wt[:, h0:h1],
                        in_=wts[:, (pc0 + h0) * MCOLS: (pc0 + h1) * MCOLS]
                        .rearrange("p (n m) -> p n m", m=MCOLS),
                    )
            for ci, bl in enumerate(chunks):
                for bi in bl:
                    for (gg, off, ln) in bins[bi]:
                        pieces = grp_pieces[gg]
                        if (bi, off) != (pieces[-1][0], pieces[-1][1]):
                            continue    # emit at the group's LAST piece
                        # q-major so each PSUM accumulation group (q-region)
                        # is consecutive — interleaved groups corrupt the PE
                        ps = psp.tile([128, 4, 256], f32, tag="ps")
                        for q in range(4):
                            for pi, (bi2, off2, ln2, pcol2) in enumerate(pieces):
                                ci2 = chunk_of_bin[bi2][0]
                                gc2, bl02 = gc_of[ci2]
                                wt2, pc02 = wt_of[ci2]
                                nc.tensor.matmul(
                                    out=ps[:, q, 0:MCOLS],
                                    lhsT=gc2[:, bi2 - bl02, q * 128: (q + 1) * 128],
                                    rhs=wt2[:, pcol2 - pc02, :],
                                    start=(pi == 0), stop=(pi == len(pieces) - 1),
                                )
                        # slot complete -> pool
                        i = ndone
                        ndone += 1
                        if i % 4 == 0:
                            f2 = fp.tile([128, 2, 2, 4, 49], f16, tag="f2")
                        fslice = f2[:, (i % 4) // 2, i % 2]
                        path = PATH_CYCLE[i % len(PATH_CYCLE)]
                        if path == "TR":
                            # single DVE op: (di,dj) cols sit at stride 49,
                            # reduce the innermost [4] window psum -> f16
                            nc.vector.tensor_reduce(
                                out=fslice,
                                in_=ps[:, :, 0:196].rearrange(
                                    "p q (d s) -> p q s d", d=4),
                                axis=mybir.AxisListType.X, op=mmax)
                        else:
                            # ACT evacuates all 196 cols; DVE folds in 4x f16
                            vt = sbp.tile([128, 4, 2, 98], f16, tag="vt")
                            nc.scalar.copy(out=vt[:], in_=ps[:, :, 0:196])
                            ft1 = sbp.tile([128, 4, 98], f16, tag="ft1")
                            nc.vector.tensor_tensor(
                                out=ft1[:], in0=vt[:, :, 0], in1=vt[:, :, 1], op=mmax)
                            nc.vector.tensor_tensor(
                                out=fslice, in0=ft1[:, :, 0:49], in1=ft1[:, :, 49:98], op=mmax)
                        if i % 4 == 3:
                            p0 = (i - 3) // 2
                            nc.sync.dma_start(
                                out=out[p0: p0 + 2]
                                .rearrange("g p (s q i) -> p g s q i", s=2, q=4),
                                in_=f2[:],
                            )
    nc.compile()
    _NC_CACHE[key] = nc
    return nc


def _prepare(bottom, rois, im_info):
    g = _grid_geometry(rois, im_info)
    pts = _roi_pts(g)
    order = np.argsort(pts, kind="stable")
    sizes = [int(pts[order[8 * gg + 7]]) for gg in range(NGROUP)]
    bins, grp_pieces, slot_order = _plan_structure(sizes)
    totp = sum(len(v) for v in grp_pieces.values())
    # cores[c][slot] = ROI id; slot s <-> group slot_order[s]
    cores = [[int(order[8 * gg + c]) for gg in slot_order] for c in range(NCORES)]
    flat16 = np.ascontiguousarray(np.asarray(bottom, np.float32).reshape(B * H * W, C).astype(np.float16))
    in_maps = []
    for c in range(NCORES):
        roi_of_group = [int(order[8 * gg + c]) for gg in range(NGROUP)]
        p = _build_core_plan(g, roi_of_group, grp_pieces, len(bins), totp)
        in_maps.append({"bottom": flat16, "gidx": p["gidx"], "w": p["w"]})
    return cores, bins, grp_pieces, slot_order, in_maps


def kernel(bottom, rois, im_info):
    from concourse.bass_utils import run_bass_kernel_spmd

    cores, bins, grp_pieces, slot_order, in_maps = _prepare(bottom, rois, im_info)
    nc = _build_bass(bins, grp_pieces, slot_order)
    res = run_bass_kernel_spmd(nc, in_maps, core_ids=list(range(NCORES)))
    out = np.empty((NROI, POOL, POOL, C), np.float32)
    for c in range(NCORES):
        r = res.results[c]["out"].reshape(NR // 2, 128, 2, 4, 7, 7).astype(np.float32)
        # [pair, c128, sl, q, pi, pj] -> [slot, pi, pj, q*128+c128]
        r = r.transpose(0, 2, 4, 5, 3, 1).reshape(NR, POOL, POOL, C)
        out[np.array(cores[c])] = r
    return out
